# revision 31
# baseline (speedup 1.0000x reference)
"""Bahdanau attention scorer for Trainium2, 8-core data-parallel over batch.

scores[b, s] = v_a . tanh(W_s @ enc_outs[s, b] + W_t @ dec_out[b] + b_t)

Shapes (fixed): enc_outs (2048, 64, 512) f32, dec_out (64, 512) f32,
W_s/W_t (512, 512) f32, b_t/v_a (512,) f32 -> scores (64, 2048) f32.

Sharding: batch 64 -> 8 cores x 8 batches. Small params replicated.

Key ideas on top of the bf16 streaming baseline:
  * The attention (a) axis is PERMUTED host-side so rows are sorted by
    |v_a| ascending. The final score error contributed by enc_att[a]
    quantization is weighted by v_a^2, and the two low-|v_a| chunks
    carry only ~2% + ~11% of sum(v_a^2). Those chunks' GEMM runs in
    pure fp8e4 (DoubleRow perf mode: 2 k-tiles per instruction, 2x PE
    throughput); the two high-|v_a| chunks stay bf16. Measured rel err
    ~1.3e-2 (gate 2e-2) for ~25% less PE matmul work.
  * All W copies are pre-scaled by 32 (exact pow2) so fp8 W avoids the
    e4m3 subnormal range; the ACT tanh applies input scale 1/32.
  * Blocks are processed in PAIRS (same batch b, adjacent 512-token
    s-blocks) with [128, 1024] PSUM tiles, one tanh per (ac, pair):
    ACT's ~290 ns/instruction fixed overhead amortizes over 1024 cols
    (~1.28 us vs 2x 0.78 us), cutting ACT busy ~18%.
  * DVE does the v_a scale + chunk-sum at pair width (4 muls + 3 adds
    of [128, 1024] bf16); the PSUM->SBUF copy of the reduced scores
    moved to the otherwise-idle Pool (gpsimd) engine.
  * enc streams twice: bf16 (16.8 MB) on the SP DMA queue and fp8
    (8.4 MB) on the Pool DMA queue, so the two queues load in parallel.
  * Partition reduce stays the ones-matmul trick: a [128, 128] all-ones
    stationary writes a full-height PSUM tile (streams faster than a
    [1, n] output); row 0 holds the scores.
"""

import sys

sys.path.insert(0, "/opt/trn_rl_repo")

import numpy as np
import ml_dtypes

import concourse.bass as bass
import concourse.mybir as mybir
import concourse.tile as tile
from concourse import bacc
from concourse.bass_utils import run_bass_kernel_spmd

S, B, H, A = 2048, 64, 512, 512
NCORES = 8
BL = B // NCORES          # local batches per core
HC = H // 128             # h chunks (k-tiles)
AC = A // 128             # a chunks
SBLK = 512                # tokens per block
NSB = S // SBLK           # s blocks per batch row
NBLK = BL * NSB           # blocks per core
NPAIR = NBLK // 2         # block pairs per core (same b, adjacent sb)
PW = 2 * SBLK             # pair width in tokens

# a-chunks (after the host-side |v_a| ascending sort) computed in pure
# fp8 DoubleRow; the rest in bf16.
FP8_AC = (0, 1)
# a-chunks whose first h-pair (k-tiles 0,1) runs fp8 DoubleRow and the
# rest bf16 (error-weight is small enough for a half-chunk fp8 bite)
HALF8_AC = (2,)
WSCALE = 32.0             # pow2 pre-scale on all W copies (undone in ACT)

F32 = mybir.dt.float32
BF16 = mybir.dt.bfloat16
FP8 = mybir.dt.float8e4
BF16_NP = ml_dtypes.bfloat16
E4M3_NP = ml_dtypes.float8_e4m3

_CACHE = {}


def build_kernel():
    nc = bacc.Bacc("TRN2", target_bir_lowering=False, debug=False,
                   num_devices=NCORES)

    nac8 = len(FP8_AC) + len(HALF8_AC)
    enc_d = nc.dram_tensor("enc", [NBLK * 128, HC * SBLK], BF16,
                           kind="ExternalInput")
    enc8_d = nc.dram_tensor("enc8", [NBLK * 128, HC, SBLK], FP8,
                            kind="ExternalInput")
    wst_d = nc.dram_tensor("wst", [128, HC * A], BF16, kind="ExternalInput")
    w8_d = nc.dram_tensor("w8", [128, nac8, HC, 128], FP8,
                          kind="ExternalInput")
    bias_d = nc.dram_tensor("bias", [128, AC * BL], F32, kind="ExternalInput")
    va_d = nc.dram_tensor("va", [128, AC], F32, kind="ExternalInput")
    out_d = nc.dram_tensor("scores", [1, BL * S], F32, kind="ExternalOutput")

    inv_scale = float(1.0 / WSCALE)

    with tile.TileContext(nc) as tc:
        with tc.tile_pool(name="consts", bufs=1) as constp:
            wst_sb = constp.tile([128, HC * A], BF16, tag="wst")
            w8_sb = constp.tile([128, nac8, HC, 128], FP8, tag="w8")
            va_sb = constp.tile([128, AC], F32, tag="va")
            bias_sb = constp.tile([128, AC * BL], F32, tag="bias")
            ones_sb = constp.tile([128, 128], BF16, tag="ones")
            nc.gpsimd.memset(ones_sb[:], 1.0)
            warm_sb = constp.tile([128, 256], BF16, tag="warm")
            nc.gpsimd.memset(warm_sb[:], 1.0)

            with (
                tc.tile_pool(name="xb", bufs=8) as xbp,        # bf16 enc
                tc.tile_pool(name="x8", bufs=8) as x8p,        # fp8 enc
                tc.tile_pool(name="act", bufs=10) as actp,     # th / vm / vacc
                tc.tile_pool(name="stage", bufs=4) as stagep,
                tc.tile_pool(name="ps_mm", bufs=3, space="PSUM") as mmp,
                tc.tile_pool(name="ps_v", bufs=1, space="PSUM") as pvp,
            ):
                pending = []  # (vacc, b, sbp) awaiting partition-reduce

                # PE p-state ramps only while busy; pre-ramp during the
                # initial DMA fill so real matmuls start near full clock.
                def warm(n):
                    for _ in range(n):
                        wps = pvp.tile([128, PW], F32, tag="pv")
                        nc.tensor.matmul(wps[:, 0:256], ones_sb[:],
                                         warm_sb[:], start=True, stop=True)

                warm(13)

                def emit_reduce(vacc, b, sbp, sync_out=False):
                    psV = pvp.tile([128, PW], F32, tag="pv")
                    for h in range(2):
                        nc.tensor.matmul(
                            psV[:, h * SBLK:(h + 1) * SBLK], ones_sb[:],
                            vacc[:, h * SBLK:(h + 1) * SBLK],
                            start=True, stop=True)
                    stg = stagep.tile([1, PW], F32, tag="stage")
                    nc.vector.tensor_copy(stg[:], psV[0:1, :])
                    nc.sync.dma_start(
                        out_d[0:1, b * S + sbp * PW: b * S + (sbp + 1) * PW],
                        stg[:])

                def emit_reduce_half(vacc_h, b, sbp, hh):
                    """512-wide reduce+copy+out for one half of the last
                    pair -- the tail drains two short chains instead of one
                    pair-wide one."""
                    psV = pvp.tile([128, PW], F32, tag="pv")
                    nc.tensor.matmul(psV[:, 0:SBLK], ones_sb[:], vacc_h[:],
                                     start=True, stop=True)
                    stg = stagep.tile([1, SBLK], F32, tag="stageh")
                    nc.vector.tensor_copy(stg[:], psV[0:1, 0:SBLK])
                    o0 = b * S + sbp * PW + hh * SBLK
                    nc.sync.dma_start(out_d[0:1, o0:o0 + SBLK], stg[:])

                def load_pair(pi, fill=False, eng=None):
                    """Issue DMAs for pair pi; returns (xb0, xb1, x80, x81)."""
                    r0 = (2 * pi) * 128
                    xb, x8 = [], []
                    for half in range(2):
                        t8 = x8p.tile([128, HC, SBLK], FP8, tag="x8")
                        nc.gpsimd.dma_start(
                            t8[:, :, :],
                            enc8_d[r0 + half * 128: r0 + (half + 1) * 128,
                                   :, :])
                        x8.append(t8)
                    for half in range(2):
                        t = xbp.tile([128, HC * SBLK], BF16, tag="xb")
                        rr = r0 + half * 128
                        if fill:
                            # per-chunk loads so fill-phase matmuls start
                            # before the whole pair is resident; block 0 on
                            # the scalar queue so it streams in parallel
                            # with the W chunks on the sync queue
                            qe = nc.scalar if half == 0 else nc.sync
                            for hcc in range(HC):
                                qe.dma_start(
                                    t[:, hcc * SBLK:(hcc + 1) * SBLK],
                                    enc_d[rr:rr + 128,
                                          hcc * SBLK:(hcc + 1) * SBLK])
                        else:
                            (eng or nc.sync).dma_start(
                                t[:], enc_d[rr:rr + 128, :])
                        xb.append(t)
                    return xb, x8

                def do_pair(b, sbp, xb, x8, fill=False, last=False):
                    """All compute for pair (b, sbp); tiles already loading."""
                    vacc = None
                    # bf16 (high |v_a|) chunks first: their tiles prefetch
                    # on the fatter queue a full pair ahead; fp8 after.
                    # For the LAST pair, fp8 first: the tail then drains a
                    # short bf16 group's ACT/DVE chain instead of waiting
                    # on back-to-back fp8 groups' activations. The FILL pair
                    # is also fp8-first: its fp8 tiles ride the short
                    # gpsimd-queue and arrive ~4us before the bf16 chunks.
                    bf = [a for a in range(AC) if a not in FP8_AC]
                    f8 = list(FP8_AC)
                    ac_order = f8 + bf if last else bf + f8
                    for gi, ac in enumerate(ac_order):
                        ps = mmp.tile([128, PW], F32, tag="mm")
                        if ac in FP8_AC:
                            ai = FP8_AC.index(ac)
                            for half in range(2):
                                for pr in range(HC // 2):
                                    nc.tensor.matmul(
                                        ps[:, half * SBLK:(half + 1) * SBLK],
                                        w8_sb[:, ai, 2 * pr:2 * pr + 2, :],
                                        x8[half][:, 2 * pr:2 * pr + 2, :],
                                        start=(pr == 0),
                                        stop=(pr == HC // 2 - 1),
                                        perf_mode=mybir.MatmulPerfMode.
                                        DoubleRow)
                        elif ac in HALF8_AC:
                            # first h-pair fp8 DoubleRow, rest bf16, one
                            # accumulation group (all W copies share the
                            # same pow2 pre-scale so PSUM units match)
                            ai = len(FP8_AC) + HALF8_AC.index(ac)
                            for half in range(2):
                                nc.tensor.matmul(
                                    ps[:, half * SBLK:(half + 1) * SBLK],
                                    w8_sb[:, ai, 0:2, :],
                                    x8[half][:, 0:2, :],
                                    start=True, stop=False,
                                    perf_mode=mybir.MatmulPerfMode.DoubleRow,
                                    skip_group_check=True)
                                for hcc in range(HC // 2, HC):
                                    nc.tensor.matmul(
                                        ps[:, half * SBLK:(half + 1) * SBLK],
                                        wst_sb[:, hcc * A + ac * 128:
                                               hcc * A + ac * 128 + 128],
                                        xb[half][:, hcc * SBLK:
                                                 (hcc + 1) * SBLK],
                                        start=False, stop=(hcc == HC - 1),
                                        skip_group_check=True)
                        else:
                            for half in range(2):
                                for hcc in range(HC):
                                    nc.tensor.matmul(
                                        ps[:, half * SBLK:(half + 1) * SBLK],
                                        wst_sb[:, hcc * A + ac * 128:
                                               hcc * A + ac * 128 + 128],
                                        xb[half][:, hcc * SBLK:
                                                 (hcc + 1) * SBLK],
                                        start=(hcc == 0),
                                        stop=(hcc == HC - 1))
                        if fill and gi == 0:
                            warm(4)
                        if gi == 2 and pending:
                            # previous pair's partition reduce, emitted once
                            # three matmul groups are queued ahead of it
                            emit_reduce(*pending.pop(0))
                        bias_ap = bias_sb[:, ac * BL + b: ac * BL + b + 1]
                        va_ap = va_sb[:, ac:ac + 1]
                        if last and gi == AC - 1:
                            # final group of the run: two independent
                            # 512-wide ACT/DVE/reduce/out chains so the tail
                            # drains short half-chains instead of one
                            # pair-wide one
                            for hh in range(2):
                                sl = slice(hh * SBLK, (hh + 1) * SBLK)
                                th = actp.tile([128, SBLK], BF16, tag="th")
                                nc.scalar.activation(
                                    th[:], ps[:, sl],
                                    mybir.ActivationFunctionType.Tanh,
                                    bias=bias_ap, scale=inv_scale)
                                vm = actp.tile([128, SBLK], BF16, tag="vm")
                                nc.vector.tensor_scalar_mul(
                                    vm[:], th[:], va_ap)
                                vh = actp.tile([128, SBLK], BF16, tag="vh")
                                nc.vector.tensor_add(
                                    vh[:], vacc[:, sl], vm[:])
                                emit_reduce_half(vh, b, sbp, hh)
                            return
                        th = actp.tile([128, PW], BF16, tag="th")
                        nc.scalar.activation(
                            th[:], ps[:],
                            mybir.ActivationFunctionType.Tanh,
                            bias=bias_ap, scale=inv_scale)
                        vm = actp.tile([128, PW], BF16, tag="vm")
                        nc.vector.tensor_scalar_mul(vm[:], th[:], va_ap)
                        if vacc is None:
                            vacc = vm
                        else:
                            # fresh output tile: an in-place add costs ~20%
                            # more on DVE than a non-aliased one
                            nv = actp.tile([128, PW], BF16, tag="vacc")
                            nc.vector.tensor_add(nv[:], vacc[:], vm[:])
                            vacc = nv
                    pending.append((vacc, b, sbp))

                # ---- fill phase: pair 0 with per-chunk loads, W chunks
                # interleaved so the first matmul starts ~0.6us in ----
                nc.gpsimd.dma_start(w8_sb[:, :, :, :], w8_d[:, :, :, :])
                nc.scalar.dma_start(va_sb[:], va_d[:, :])
                nc.scalar.dma_start(bias_sb[:], bias_d[:, :])
                for hcc in range(HC):
                    nc.sync.dma_start(wst_sb[:, hcc * A:(hcc + 1) * A],
                                      wst_d[:, hcc * A:(hcc + 1) * A])

                # pairs 1-2 load via the scalar queue: the sync queue is
                # already committed to the W + fill chunks, and ACT has no
                # work yet so its sequencer issues these for free
                xb, x8 = load_pair(0, fill=True)
                pref = [load_pair(1, eng=nc.scalar),
                        load_pair(2, eng=nc.scalar),
                        load_pair(3)]
                do_pair(0, 0, xb, x8, fill=True)

                for pi in range(1, NPAIR):
                    if pi + 3 < NPAIR:
                        pref.append(load_pair(pi + 3))
                    b, sbp = divmod(pi, NSB // 2)
                    do_pair(b, sbp, *pref.pop(0), last=(pi == NPAIR - 1))

                while pending:
                    emit_reduce(*pending.pop(0))

    nc.compile()
    return nc


def _prep_host(dec_out, enc_outs, W_s, W_t, b_t, v_a):
    nac8 = len(FP8_AC) + len(HALF8_AC)
    # sort attention rows by |v_a| ascending (free relabeling of a)
    perm = np.argsort(np.abs(v_a), kind="stable")
    Wp = W_s[perm].astype(np.float64) * WSCALE
    vap = v_a[perm]
    # dec bias, exact on host: bias[a, b] = (W_t @ dec[b] + b_t)[a], permuted
    bias = (dec_out.astype(np.float64) @ W_t[perm].T.astype(np.float64)
            + b_t[perm].astype(np.float64)).T.astype(np.float32)   # (A, B)

    # W_s.T (scaled) as [128 h-part, HC * A] bf16
    wst = np.ascontiguousarray(
        Wp.T.reshape(HC, 128, A).transpose(1, 0, 2).reshape(128, HC * A)
    ).astype(BF16_NP)
    # fp8 W per fp8 a-chunk: [128 h-part, nac8, HC, 128 a]
    w8 = np.zeros((128, nac8, HC, 128), dtype=np.float64)
    for ai, ac in enumerate(tuple(FP8_AC) + tuple(HALF8_AC)):
        blk = Wp[ac * 128:(ac + 1) * 128, :]          # (128 a, H)
        w8[:, ai, :, :] = blk.T.reshape(HC, 128, 128).transpose(1, 0, 2)
    w8 = np.ascontiguousarray(w8).astype(E4M3_NP)

    va4 = np.ascontiguousarray(
        vap.reshape(AC, 128).T).astype(np.float32)           # (128, AC)

    enc_bf = enc_outs.astype(BF16_NP)                        # (S, B, H)
    enc_f8 = enc_outs.astype(E4M3_NP)
    in_maps = []
    for k in range(NCORES):
        # -> [b, sb, p, hc, c] -> row (b*NSB+sb)*128 + p, col hc*SBLK + c
        e = enc_bf[:, k * BL:(k + 1) * BL, :]
        e6 = e.reshape(NSB, SBLK, BL, HC, 128).transpose(2, 0, 4, 3, 1)
        enc_l = np.ascontiguousarray(e6).reshape(NBLK * 128, HC * SBLK)
        e8 = enc_f8[:, k * BL:(k + 1) * BL, :]
        e86 = e8.reshape(NSB, SBLK, BL, HC, 128).transpose(2, 0, 4, 3, 1)
        enc8_l = np.ascontiguousarray(e86).reshape(NBLK * 128, HC, SBLK)
        bl = bias[:, k * BL:(k + 1) * BL]                    # (A, BL)
        bias_l = np.ascontiguousarray(
            bl.reshape(AC, 128, BL).transpose(1, 0, 2).reshape(128, AC * BL))
        in_maps.append({
            "enc": enc_l,
            "enc8": enc8_l,
            "wst": wst,
            "w8": w8,
            "bias": bias_l,
            "va": va4,
        })
    return in_maps


def kernel(dec_out, enc_outs, W_s, W_t, b_t, v_a, trace=False):
    dec_out = np.asarray(dec_out)
    enc_outs = np.asarray(enc_outs)
    if "nc" not in _CACHE:
        _CACHE["nc"] = build_kernel()
    nc = _CACHE["nc"]
    in_maps = _prep_host(dec_out, enc_outs,
                         np.asarray(W_s), np.asarray(W_t),
                         np.asarray(b_t), np.asarray(v_a))
    res = run_bass_kernel_spmd(nc, in_maps, core_ids=list(range(NCORES)),
                               trace=trace)
    out = np.concatenate(
        [res.results[k]["scores"].reshape(BL, S) for k in range(NCORES)],
        axis=0).astype(np.float32)
    if trace:
        _CACHE["last_result"] = res
    return out


# revision 32
# speedup vs baseline: 1.0312x; 1.0312x over previous
"""Bahdanau attention scorer for Trainium2, 8-core data-parallel over batch.

scores[b, s] = v_a . tanh(W_s @ enc_outs[s, b] + W_t @ dec_out[b] + b_t)

Shapes (fixed): enc_outs (2048, 64, 512) f32, dec_out (64, 512) f32,
W_s/W_t (512, 512) f32, b_t/v_a (512,) f32 -> scores (64, 2048) f32.

Sharding: batch 64 -> 8 cores x 8 batches. Small params replicated.

Key ideas on top of the bf16 streaming baseline:
  * The attention (a) axis is PERMUTED host-side so rows are sorted by
    |v_a| ascending. The final score error contributed by enc_att[a]
    quantization is weighted by v_a^2, and the two low-|v_a| chunks
    carry only ~2% + ~11% of sum(v_a^2). Those chunks' GEMM runs in
    pure fp8e4 (DoubleRow perf mode: 2 k-tiles per instruction, 2x PE
    throughput); the two high-|v_a| chunks stay bf16. Measured rel err
    ~1.3e-2 (gate 2e-2) for ~25% less PE matmul work.
  * All W copies are pre-scaled by 32 (exact pow2) so fp8 W avoids the
    e4m3 subnormal range; the ACT tanh applies input scale 1/32.
  * Blocks are processed in PAIRS (same batch b, adjacent 512-token
    s-blocks) with [128, 1024] PSUM tiles, one tanh per (ac, pair):
    ACT's ~290 ns/instruction fixed overhead amortizes over 1024 cols
    (~1.28 us vs 2x 0.78 us), cutting ACT busy ~18%.
  * DVE does the v_a scale + chunk-sum at pair width (4 muls + 3 adds
    of [128, 1024] bf16); the PSUM->SBUF copy of the reduced scores
    moved to the otherwise-idle Pool (gpsimd) engine.
  * enc streams twice: bf16 (16.8 MB) on the SP DMA queue and fp8
    (8.4 MB) on the Pool DMA queue, so the two queues load in parallel.
  * Partition reduce stays the ones-matmul trick: a [128, 128] all-ones
    stationary writes a full-height PSUM tile (streams faster than a
    [1, n] output); row 0 holds the scores.
"""

import sys

sys.path.insert(0, "/opt/trn_rl_repo")

import numpy as np
import ml_dtypes

import concourse.bass as bass
import concourse.mybir as mybir
import concourse.tile as tile
from concourse import bacc
from concourse.bass_utils import run_bass_kernel_spmd

S, B, H, A = 2048, 64, 512, 512
NCORES = 8
BL = B // NCORES          # local batches per core
HC = H // 128             # h chunks (k-tiles)
AC = A // 128             # a chunks
SBLK = 512                # tokens per block
NSB = S // SBLK           # s blocks per batch row
NBLK = BL * NSB           # blocks per core
NPAIR = NBLK // 2         # block pairs per core (same b, adjacent sb)
PW = 2 * SBLK             # pair width in tokens

# a-chunks (after the host-side |v_a| ascending sort) computed in pure
# fp8 DoubleRow; the rest in bf16.
FP8_AC = (0, 1)
# a-chunks whose first h-pair (k-tiles 0,1) runs fp8 DoubleRow and the
# rest bf16 (error-weight is small enough for a half-chunk fp8 bite)
HALF8_AC = (2,)
WSCALE = 32.0             # pow2 pre-scale on all W copies (undone in ACT)

F32 = mybir.dt.float32
BF16 = mybir.dt.bfloat16
FP8 = mybir.dt.float8e4
BF16_NP = ml_dtypes.bfloat16
E4M3_NP = ml_dtypes.float8_e4m3

_CACHE = {}


def build_kernel():
    nc = bacc.Bacc("TRN2", target_bir_lowering=False, debug=False,
                   num_devices=NCORES)

    nac8 = len(FP8_AC) + len(HALF8_AC)
    enc_d = nc.dram_tensor("enc", [NBLK * 128, HC * SBLK], BF16,
                           kind="ExternalInput")
    enc8_d = nc.dram_tensor("enc8", [NBLK * 128, HC, SBLK], FP8,
                            kind="ExternalInput")
    wst_d = nc.dram_tensor("wst", [128, HC * A], BF16, kind="ExternalInput")
    w8_d = nc.dram_tensor("w8", [128, nac8, HC, 128], FP8,
                          kind="ExternalInput")
    bias_d = nc.dram_tensor("bias", [128, AC * BL], F32, kind="ExternalInput")
    va_d = nc.dram_tensor("va", [128, AC], F32, kind="ExternalInput")
    out_d = nc.dram_tensor("scores", [1, BL * S], F32, kind="ExternalOutput")

    inv_scale = float(1.0 / WSCALE)

    with tile.TileContext(nc) as tc:
        with tc.tile_pool(name="consts", bufs=1) as constp:
            wst_sb = constp.tile([128, HC * A], BF16, tag="wst")
            w8_sb = constp.tile([128, nac8, HC, 128], FP8, tag="w8")
            va_sb = constp.tile([128, AC], F32, tag="va")
            bias_sb = constp.tile([128, AC * BL], F32, tag="bias")
            ones_sb = constp.tile([128, 128], BF16, tag="ones")
            nc.gpsimd.memset(ones_sb[:], 1.0)
            warm_sb = constp.tile([128, 256], BF16, tag="warm")
            nc.gpsimd.memset(warm_sb[:], 1.0)

            with (
                tc.tile_pool(name="xb", bufs=6) as xbp,        # bf16 enc
                tc.tile_pool(name="x8", bufs=6) as x8p,        # fp8 enc
                tc.tile_pool(name="act", bufs=10) as actp,     # th / vm / vacc
                tc.tile_pool(name="stage", bufs=4) as stagep,
                tc.tile_pool(name="ps_mm", bufs=3, space="PSUM") as mmp,
                tc.tile_pool(name="ps_v", bufs=1, space="PSUM") as pvp,
            ):
                pending = []  # (vacc, b, sbp) awaiting partition-reduce

                # PE p-state ramps only while busy; pre-ramp during the
                # initial DMA fill so real matmuls start near full clock.
                def warm(n):
                    for _ in range(n):
                        wps = pvp.tile([128, PW], F32, tag="pv")
                        nc.tensor.matmul(wps[:, 0:256], ones_sb[:],
                                         warm_sb[:], start=True, stop=True)

                warm(13)

                def emit_reduce(vacc, b, sbp, sync_out=False):
                    psV = pvp.tile([128, PW], F32, tag="pv")
                    for h in range(2):
                        nc.tensor.matmul(
                            psV[:, h * SBLK:(h + 1) * SBLK], ones_sb[:],
                            vacc[:, h * SBLK:(h + 1) * SBLK],
                            start=True, stop=True)
                    stg = stagep.tile([1, PW], F32, tag="stage")
                    nc.vector.tensor_copy(stg[:], psV[0:1, :])
                    nc.sync.dma_start(
                        out_d[0:1, b * S + sbp * PW: b * S + (sbp + 1) * PW],
                        stg[:])

                def emit_reduce_half(vacc_h, b, sbp, hh):
                    """512-wide reduce+copy+out for one half of the last
                    pair -- the tail drains two short chains instead of one
                    pair-wide one."""
                    psV = pvp.tile([128, PW], F32, tag="pv")
                    nc.tensor.matmul(psV[:, 0:SBLK], ones_sb[:], vacc_h[:],
                                     start=True, stop=True)
                    stg = stagep.tile([1, SBLK], F32, tag="stageh")
                    nc.vector.tensor_copy(stg[:], psV[0:1, 0:SBLK])
                    o0 = b * S + sbp * PW + hh * SBLK
                    nc.sync.dma_start(out_d[0:1, o0:o0 + SBLK], stg[:])

                def load_pair(pi, fill=False, eng=None):
                    """Issue DMAs for pair pi; returns (xb0, xb1, x80, x81)."""
                    r0 = (2 * pi) * 128
                    xb, x8 = [], []
                    for half in range(2):
                        t8 = x8p.tile([128, HC, SBLK], FP8, tag="x8")
                        nc.gpsimd.dma_start(
                            t8[:, :, :],
                            enc8_d[r0 + half * 128: r0 + (half + 1) * 128,
                                   :, :])
                        x8.append(t8)
                    for half in range(2):
                        t = xbp.tile([128, HC * SBLK], BF16, tag="xb")
                        rr = r0 + half * 128
                        if fill:
                            # per-chunk loads so fill-phase matmuls start
                            # before the whole pair is resident; block 0 on
                            # the scalar queue so it streams in parallel
                            # with the W chunks on the sync queue
                            qe = nc.scalar if half == 0 else nc.sync
                            for hcc in range(HC):
                                qe.dma_start(
                                    t[:, hcc * SBLK:(hcc + 1) * SBLK],
                                    enc_d[rr:rr + 128,
                                          hcc * SBLK:(hcc + 1) * SBLK])
                        else:
                            (eng or nc.sync).dma_start(
                                t[:], enc_d[rr:rr + 128, :])
                        xb.append(t)
                    return xb, x8

                def do_pair(b, sbp, xb, x8, fill=False, last=False):
                    """All compute for pair (b, sbp); tiles already loading."""
                    vacc = None
                    # bf16 (high |v_a|) chunks first: their tiles prefetch
                    # on the fatter queue a full pair ahead; fp8 after.
                    # For the LAST pair, fp8 first: the tail then drains a
                    # short bf16 group's ACT/DVE chain instead of waiting
                    # on back-to-back fp8 groups' activations. The FILL pair
                    # is also fp8-first: its fp8 tiles ride the short
                    # gpsimd-queue and arrive ~4us before the bf16 chunks.
                    bf = [a for a in range(AC) if a not in FP8_AC]
                    f8 = list(FP8_AC)
                    ac_order = f8 + bf if last else bf + f8
                    for gi, ac in enumerate(ac_order):
                        ps = mmp.tile([128, PW], F32, tag="mm")
                        if ac in FP8_AC:
                            ai = FP8_AC.index(ac)
                            for half in range(2):
                                for pr in range(HC // 2):
                                    nc.tensor.matmul(
                                        ps[:, half * SBLK:(half + 1) * SBLK],
                                        w8_sb[:, ai, 2 * pr:2 * pr + 2, :],
                                        x8[half][:, 2 * pr:2 * pr + 2, :],
                                        start=(pr == 0),
                                        stop=(pr == HC // 2 - 1),
                                        perf_mode=mybir.MatmulPerfMode.
                                        DoubleRow)
                        elif ac in HALF8_AC:
                            # first h-pair fp8 DoubleRow, rest bf16, one
                            # accumulation group (all W copies share the
                            # same pow2 pre-scale so PSUM units match)
                            ai = len(FP8_AC) + HALF8_AC.index(ac)
                            for half in range(2):
                                nc.tensor.matmul(
                                    ps[:, half * SBLK:(half + 1) * SBLK],
                                    w8_sb[:, ai, 0:2, :],
                                    x8[half][:, 0:2, :],
                                    start=True, stop=False,
                                    perf_mode=mybir.MatmulPerfMode.DoubleRow,
                                    skip_group_check=True)
                                for hcc in range(HC // 2, HC):
                                    nc.tensor.matmul(
                                        ps[:, half * SBLK:(half + 1) * SBLK],
                                        wst_sb[:, hcc * A + ac * 128:
                                               hcc * A + ac * 128 + 128],
                                        xb[half][:, hcc * SBLK:
                                                 (hcc + 1) * SBLK],
                                        start=False, stop=(hcc == HC - 1),
                                        skip_group_check=True)
                        else:
                            for half in range(2):
                                for hcc in range(HC):
                                    nc.tensor.matmul(
                                        ps[:, half * SBLK:(half + 1) * SBLK],
                                        wst_sb[:, hcc * A + ac * 128:
                                               hcc * A + ac * 128 + 128],
                                        xb[half][:, hcc * SBLK:
                                                 (hcc + 1) * SBLK],
                                        start=(hcc == 0),
                                        stop=(hcc == HC - 1))
                        if fill and gi == 0:
                            warm(4)
                        if gi == 2 and pending:
                            # previous pair's partition reduce, emitted once
                            # three matmul groups are queued ahead of it
                            emit_reduce(*pending.pop(0))
                        bias_ap = bias_sb[:, ac * BL + b: ac * BL + b + 1]
                        va_ap = va_sb[:, ac:ac + 1]
                        if last and gi == AC - 1:
                            # final group of the run: two independent
                            # 512-wide ACT/DVE/reduce/out chains so the tail
                            # drains short half-chains instead of one
                            # pair-wide one
                            for hh in range(2):
                                sl = slice(hh * SBLK, (hh + 1) * SBLK)
                                th = actp.tile([128, SBLK], BF16, tag="th")
                                nc.scalar.activation(
                                    th[:], ps[:, sl],
                                    mybir.ActivationFunctionType.Tanh,
                                    bias=bias_ap, scale=inv_scale)
                                vm = actp.tile([128, SBLK], BF16, tag="vm")
                                nc.vector.tensor_scalar_mul(
                                    vm[:], th[:], va_ap)
                                vh = actp.tile([128, SBLK], BF16, tag="vh")
                                nc.vector.tensor_add(
                                    vh[:], vacc[:, sl], vm[:])
                                emit_reduce_half(vh, b, sbp, hh)
                            return
                        th = actp.tile([128, PW], BF16, tag="th")
                        nc.scalar.activation(
                            th[:], ps[:],
                            mybir.ActivationFunctionType.Tanh,
                            bias=bias_ap, scale=inv_scale)
                        vm = actp.tile([128, PW], BF16, tag="vm")
                        nc.vector.tensor_scalar_mul(vm[:], th[:], va_ap)
                        if vacc is None:
                            vacc = vm
                        else:
                            # fresh output tile: an in-place add costs ~20%
                            # more on DVE than a non-aliased one
                            nv = actp.tile([128, PW], BF16, tag="vacc")
                            nc.vector.tensor_add(nv[:], vacc[:], vm[:])
                            vacc = nv
                    pending.append((vacc, b, sbp))

                # ---- fill phase: pair 0 with per-chunk loads, W chunks
                # interleaved so the first matmul starts ~0.6us in ----
                nc.gpsimd.dma_start(w8_sb[:, :, :, :], w8_d[:, :, :, :])
                nc.scalar.dma_start(va_sb[:], va_d[:, :])
                nc.scalar.dma_start(bias_sb[:], bias_d[:, :])
                for hcc in range(HC):
                    nc.sync.dma_start(wst_sb[:, hcc * A:(hcc + 1) * A],
                                      wst_d[:, hcc * A:(hcc + 1) * A])

                # pairs 1-2 load via the scalar queue: the sync queue is
                # already committed to the W + fill chunks, and ACT has no
                # work yet so its sequencer issues these for free
                xb, x8 = load_pair(0, fill=True)
                pref = [load_pair(1, eng=nc.scalar),
                        load_pair(2, eng=nc.scalar)]
                do_pair(0, 0, xb, x8, fill=True)

                for pi in range(1, NPAIR):
                    if pi + 2 < NPAIR:
                        pref.append(load_pair(pi + 2))
                    b, sbp = divmod(pi, NSB // 2)
                    do_pair(b, sbp, *pref.pop(0), last=(pi == NPAIR - 1))

                while pending:
                    emit_reduce(*pending.pop(0))

    nc.compile()
    return nc


def _prep_host(dec_out, enc_outs, W_s, W_t, b_t, v_a):
    nac8 = len(FP8_AC) + len(HALF8_AC)
    # sort attention rows by |v_a| ascending (free relabeling of a)
    perm = np.argsort(np.abs(v_a), kind="stable")
    Wp = W_s[perm].astype(np.float64) * WSCALE
    vap = v_a[perm]
    # dec bias, exact on host: bias[a, b] = (W_t @ dec[b] + b_t)[a], permuted
    bias = (dec_out.astype(np.float64) @ W_t[perm].T.astype(np.float64)
            + b_t[perm].astype(np.float64)).T.astype(np.float32)   # (A, B)

    # W_s.T (scaled) as [128 h-part, HC * A] bf16
    wst = np.ascontiguousarray(
        Wp.T.reshape(HC, 128, A).transpose(1, 0, 2).reshape(128, HC * A)
    ).astype(BF16_NP)
    # fp8 W per fp8 a-chunk: [128 h-part, nac8, HC, 128 a]
    w8 = np.zeros((128, nac8, HC, 128), dtype=np.float64)
    for ai, ac in enumerate(tuple(FP8_AC) + tuple(HALF8_AC)):
        blk = Wp[ac * 128:(ac + 1) * 128, :]          # (128 a, H)
        w8[:, ai, :, :] = blk.T.reshape(HC, 128, 128).transpose(1, 0, 2)
    w8 = np.ascontiguousarray(w8).astype(E4M3_NP)

    va4 = np.ascontiguousarray(
        vap.reshape(AC, 128).T).astype(np.float32)           # (128, AC)

    enc_bf = enc_outs.astype(BF16_NP)                        # (S, B, H)
    enc_f8 = enc_outs.astype(E4M3_NP)
    in_maps = []
    for k in range(NCORES):
        # -> [b, sb, p, hc, c] -> row (b*NSB+sb)*128 + p, col hc*SBLK + c
        e = enc_bf[:, k * BL:(k + 1) * BL, :]
        e6 = e.reshape(NSB, SBLK, BL, HC, 128).transpose(2, 0, 4, 3, 1)
        enc_l = np.ascontiguousarray(e6).reshape(NBLK * 128, HC * SBLK)
        e8 = enc_f8[:, k * BL:(k + 1) * BL, :]
        e86 = e8.reshape(NSB, SBLK, BL, HC, 128).transpose(2, 0, 4, 3, 1)
        enc8_l = np.ascontiguousarray(e86).reshape(NBLK * 128, HC, SBLK)
        bl = bias[:, k * BL:(k + 1) * BL]                    # (A, BL)
        bias_l = np.ascontiguousarray(
            bl.reshape(AC, 128, BL).transpose(1, 0, 2).reshape(128, AC * BL))
        in_maps.append({
            "enc": enc_l,
            "enc8": enc8_l,
            "wst": wst,
            "w8": w8,
            "bias": bias_l,
            "va": va4,
        })
    return in_maps


def kernel(dec_out, enc_outs, W_s, W_t, b_t, v_a, trace=False):
    dec_out = np.asarray(dec_out)
    enc_outs = np.asarray(enc_outs)
    if "nc" not in _CACHE:
        _CACHE["nc"] = build_kernel()
    nc = _CACHE["nc"]
    in_maps = _prep_host(dec_out, enc_outs,
                         np.asarray(W_s), np.asarray(W_t),
                         np.asarray(b_t), np.asarray(v_a))
    res = run_bass_kernel_spmd(nc, in_maps, core_ids=list(range(NCORES)),
                               trace=trace)
    out = np.concatenate(
        [res.results[k]["scores"].reshape(BL, S) for k in range(NCORES)],
        axis=0).astype(np.float32)
    if trace:
        _CACHE["last_result"] = res
    return out


# revision 36
# speedup vs baseline: 1.0383x; 1.0068x over previous
"""Bahdanau attention scorer for Trainium2, 8-core data-parallel over batch.

scores[b, s] = v_a . tanh(W_s @ enc_outs[s, b] + W_t @ dec_out[b] + b_t)

Shapes (fixed): enc_outs (2048, 64, 512) f32, dec_out (64, 512) f32,
W_s/W_t (512, 512) f32, b_t/v_a (512,) f32 -> scores (64, 2048) f32.

Sharding: batch 64 -> 8 cores x 8 batches. Small params replicated.

Key ideas on top of the bf16 streaming baseline:
  * The attention (a) axis is PERMUTED host-side so rows are sorted by
    |v_a| ascending. The final score error contributed by enc_att[a]
    quantization is weighted by v_a^2, and the two low-|v_a| chunks
    carry only ~2% + ~11% of sum(v_a^2). Those chunks' GEMM runs in
    pure fp8e4 (DoubleRow perf mode: 2 k-tiles per instruction, 2x PE
    throughput); the two high-|v_a| chunks stay bf16. Measured rel err
    ~1.3e-2 (gate 2e-2) for ~25% less PE matmul work.
  * All W copies are pre-scaled by 32 (exact pow2) so fp8 W avoids the
    e4m3 subnormal range; the ACT tanh applies input scale 1/32.
  * Blocks are processed in PAIRS (same batch b, adjacent 512-token
    s-blocks) with [128, 1024] PSUM tiles, one tanh per (ac, pair):
    ACT's ~290 ns/instruction fixed overhead amortizes over 1024 cols
    (~1.28 us vs 2x 0.78 us), cutting ACT busy ~18%.
  * DVE does the v_a scale + chunk-sum at pair width (4 muls + 3 adds
    of [128, 1024] bf16); the PSUM->SBUF copy of the reduced scores
    moved to the otherwise-idle Pool (gpsimd) engine.
  * enc streams twice: bf16 (16.8 MB) on the SP DMA queue and fp8
    (8.4 MB) on the Pool DMA queue, so the two queues load in parallel.
  * Partition reduce stays the ones-matmul trick: a [128, 128] all-ones
    stationary writes a full-height PSUM tile (streams faster than a
    [1, n] output); row 0 holds the scores.
"""

import sys

sys.path.insert(0, "/opt/trn_rl_repo")

import numpy as np
import ml_dtypes

import concourse.bass as bass
import concourse.mybir as mybir
import concourse.tile as tile
from concourse import bacc
from concourse.bass_utils import run_bass_kernel_spmd

S, B, H, A = 2048, 64, 512, 512
NCORES = 8
BL = B // NCORES          # local batches per core
HC = H // 128             # h chunks (k-tiles)
AC = A // 128             # a chunks
SBLK = 512                # tokens per block
NSB = S // SBLK           # s blocks per batch row
NBLK = BL * NSB           # blocks per core
NPAIR = NBLK // 2         # block pairs per core (same b, adjacent sb)
PW = 2 * SBLK             # pair width in tokens

# a-chunks (after the host-side |v_a| ascending sort) computed in pure
# fp8 DoubleRow; the rest in bf16.
FP8_AC = (0, 1)
# a-chunks whose first h-pair (k-tiles 0,1) runs fp8 DoubleRow and the
# rest bf16 (error-weight is small enough for a half-chunk fp8 bite)
HALF8_AC = (2,)
WSCALE = 32.0             # pow2 pre-scale on all W copies (undone in ACT)

F32 = mybir.dt.float32
BF16 = mybir.dt.bfloat16
FP8 = mybir.dt.float8e4
BF16_NP = ml_dtypes.bfloat16
E4M3_NP = ml_dtypes.float8_e4m3

_CACHE = {}


def build_kernel():
    nc = bacc.Bacc("TRN2", target_bir_lowering=False, debug=False,
                   num_devices=NCORES)

    nac8 = len(FP8_AC) + len(HALF8_AC)
    enc_d = nc.dram_tensor("enc", [NBLK * 128, HC * SBLK], BF16,
                           kind="ExternalInput")
    enc8_d = nc.dram_tensor("enc8", [NBLK * 128, HC, SBLK], FP8,
                            kind="ExternalInput")
    wst_d = nc.dram_tensor("wst", [128, HC * A], BF16, kind="ExternalInput")
    w8_d = nc.dram_tensor("w8", [128, nac8, HC, 128], FP8,
                          kind="ExternalInput")
    bias_d = nc.dram_tensor("bias", [128, AC * BL], F32, kind="ExternalInput")
    va_d = nc.dram_tensor("va", [128, AC], F32, kind="ExternalInput")
    out_d = nc.dram_tensor("scores", [1, BL * S], F32, kind="ExternalOutput")

    inv_scale = float(1.0 / WSCALE)

    with tile.TileContext(nc) as tc:
        with tc.tile_pool(name="consts", bufs=1) as constp:
            wst_sb = constp.tile([128, HC * A], BF16, tag="wst")
            w8_sb = constp.tile([128, nac8, HC, 128], FP8, tag="w8")
            va_sb = constp.tile([128, AC], F32, tag="va")
            bias_sb = constp.tile([128, AC * BL], F32, tag="bias")
            ones_sb = constp.tile([128, 128], BF16, tag="ones")
            nc.gpsimd.memset(ones_sb[:], 1.0)
            warm_sb = constp.tile([128, 256], BF16, tag="warm")
            nc.gpsimd.memset(warm_sb[:], 1.0)

            with (
                tc.tile_pool(name="xb", bufs=6) as xbp,        # bf16 enc
                tc.tile_pool(name="x8", bufs=6) as x8p,        # fp8 enc
                tc.tile_pool(name="act", bufs=10) as actp,     # th / vm / vacc
                tc.tile_pool(name="stage", bufs=4) as stagep,
                tc.tile_pool(name="ps_mm", bufs=3, space="PSUM") as mmp,
                tc.tile_pool(name="ps_v", bufs=1, space="PSUM") as pvp,
            ):
                pending = []  # (vacc, b, sbp) awaiting partition-reduce

                # PE p-state ramps only while busy; pre-ramp during the
                # initial DMA fill so real matmuls start near full clock.
                def warm(n):
                    for _ in range(n):
                        wps = pvp.tile([128, PW], F32, tag="pv")
                        nc.tensor.matmul(wps[:, 0:256], ones_sb[:],
                                         warm_sb[:], start=True, stop=True)

                warm(13)

                def emit_reduce(vacc, b, sbp, sync_out=False):
                    psV = pvp.tile([128, PW], F32, tag="pv")
                    for h in range(2):
                        nc.tensor.matmul(
                            psV[:, h * SBLK:(h + 1) * SBLK], ones_sb[:],
                            vacc[:, h * SBLK:(h + 1) * SBLK],
                            start=True, stop=True)
                    stg = stagep.tile([1, PW], F32, tag="stage")
                    nc.vector.tensor_copy(stg[:], psV[0:1, :])
                    nc.sync.dma_start(
                        out_d[0:1, b * S + sbp * PW: b * S + (sbp + 1) * PW],
                        stg[:])

                def emit_reduce_half(psV, vacc_h, b, sbp, hh):
                    """512-wide reduce+copy+out for one half of the last
                    pair -- both halves share one psV tile (disjoint column
                    ranges) so their chains overlap instead of serializing
                    on a pool-rotation WAR."""
                    sl = slice(hh * SBLK, (hh + 1) * SBLK)
                    nc.tensor.matmul(psV[:, sl], ones_sb[:], vacc_h[:],
                                     start=True, stop=True)
                    stg = stagep.tile([1, SBLK], F32, tag="stageh")
                    nc.vector.tensor_copy(stg[:], psV[0:1, sl])
                    o0 = b * S + sbp * PW + hh * SBLK
                    nc.sync.dma_start(out_d[0:1, o0:o0 + SBLK], stg[:])

                def load_pair(pi, fill=False, eng=None):
                    """Issue DMAs for pair pi; returns (xb0, xb1, x80, x81)."""
                    r0 = (2 * pi) * 128
                    xb, x8 = [], []
                    for half in range(2):
                        t8 = x8p.tile([128, HC, SBLK], FP8, tag="x8")
                        nc.gpsimd.dma_start(
                            t8[:, :, :],
                            enc8_d[r0 + half * 128: r0 + (half + 1) * 128,
                                   :, :])
                        x8.append(t8)
                    for half in range(2):
                        t = xbp.tile([128, HC * SBLK], BF16, tag="xb")
                        rr = r0 + half * 128
                        if fill:
                            # per-chunk loads so fill-phase matmuls start
                            # before the whole pair is resident; block 0 on
                            # the scalar queue so it streams in parallel
                            # with the W chunks on the sync queue
                            qe = nc.scalar if half == 0 else nc.sync
                            for hcc in range(HC):
                                qe.dma_start(
                                    t[:, hcc * SBLK:(hcc + 1) * SBLK],
                                    enc_d[rr:rr + 128,
                                          hcc * SBLK:(hcc + 1) * SBLK])
                        else:
                            (eng or nc.sync).dma_start(
                                t[:], enc_d[rr:rr + 128, :])
                        xb.append(t)
                    return xb, x8

                def do_pair(b, sbp, xb, x8, fill=False, last=False):
                    """All compute for pair (b, sbp); tiles already loading."""
                    vacc = None
                    # bf16 (high |v_a|) chunks first: their tiles prefetch
                    # on the fatter queue a full pair ahead; fp8 after.
                    # For the LAST pair, fp8 first: the tail then drains a
                    # short bf16 group's ACT/DVE chain instead of waiting
                    # on back-to-back fp8 groups' activations. The FILL pair
                    # is also fp8-first: its fp8 tiles ride the short
                    # gpsimd-queue and arrive ~4us before the bf16 chunks.
                    bf = [a for a in range(AC) if a not in FP8_AC]
                    f8 = list(FP8_AC)
                    ac_order = f8 + bf if last else bf + f8
                    for gi, ac in enumerate(ac_order):
                        ps = mmp.tile([128, PW], F32, tag="mm")
                        if ac in FP8_AC:
                            ai = FP8_AC.index(ac)
                            for half in range(2):
                                for pr in range(HC // 2):
                                    nc.tensor.matmul(
                                        ps[:, half * SBLK:(half + 1) * SBLK],
                                        w8_sb[:, ai, 2 * pr:2 * pr + 2, :],
                                        x8[half][:, 2 * pr:2 * pr + 2, :],
                                        start=(pr == 0),
                                        stop=(pr == HC // 2 - 1),
                                        perf_mode=mybir.MatmulPerfMode.
                                        DoubleRow)
                        elif ac in HALF8_AC:
                            # first h-pair fp8 DoubleRow, rest bf16, one
                            # accumulation group (all W copies share the
                            # same pow2 pre-scale so PSUM units match)
                            ai = len(FP8_AC) + HALF8_AC.index(ac)
                            for half in range(2):
                                nc.tensor.matmul(
                                    ps[:, half * SBLK:(half + 1) * SBLK],
                                    w8_sb[:, ai, 0:2, :],
                                    x8[half][:, 0:2, :],
                                    start=True, stop=False,
                                    perf_mode=mybir.MatmulPerfMode.DoubleRow,
                                    skip_group_check=True)
                                for hcc in range(HC // 2, HC):
                                    nc.tensor.matmul(
                                        ps[:, half * SBLK:(half + 1) * SBLK],
                                        wst_sb[:, hcc * A + ac * 128:
                                               hcc * A + ac * 128 + 128],
                                        xb[half][:, hcc * SBLK:
                                                 (hcc + 1) * SBLK],
                                        start=False, stop=(hcc == HC - 1),
                                        skip_group_check=True)
                        else:
                            for half in range(2):
                                for hcc in range(HC):
                                    nc.tensor.matmul(
                                        ps[:, half * SBLK:(half + 1) * SBLK],
                                        wst_sb[:, hcc * A + ac * 128:
                                               hcc * A + ac * 128 + 128],
                                        xb[half][:, hcc * SBLK:
                                                 (hcc + 1) * SBLK],
                                        start=(hcc == 0),
                                        stop=(hcc == HC - 1))
                        if fill and gi == 0:
                            warm(4)
                        if gi == AC - 1 and pending:
                            # previous pair's partition reduce, emitted with
                            # most of this pair's matmuls queued ahead so the
                            # previous ACT/DVE chain has surely drained
                            emit_reduce(*pending.pop(0))
                        bias_ap = bias_sb[:, ac * BL + b: ac * BL + b + 1]
                        va_ap = va_sb[:, ac:ac + 1]
                        if last and gi == AC - 1:
                            # final group of the run: two independent
                            # 512-wide ACT/DVE/reduce/out chains so the tail
                            # drains short half-chains instead of one
                            # pair-wide one
                            psV = pvp.tile([128, PW], F32, tag="pv")
                            for hh in range(2):
                                sl = slice(hh * SBLK, (hh + 1) * SBLK)
                                th = actp.tile([128, SBLK], BF16, tag="th")
                                nc.scalar.activation(
                                    th[:], ps[:, sl],
                                    mybir.ActivationFunctionType.Tanh,
                                    bias=bias_ap, scale=inv_scale)
                                vm = actp.tile([128, SBLK], BF16, tag="vm")
                                nc.vector.tensor_scalar_mul(
                                    vm[:], th[:], va_ap)
                                vh = actp.tile([128, SBLK], BF16, tag="vh")
                                nc.vector.tensor_add(
                                    vh[:], vacc[:, sl], vm[:])
                                emit_reduce_half(psV, vh, b, sbp, hh)
                            return
                        th = actp.tile([128, PW], BF16, tag="th")
                        nc.scalar.activation(
                            th[:], ps[:],
                            mybir.ActivationFunctionType.Tanh,
                            bias=bias_ap, scale=inv_scale)
                        vm = actp.tile([128, PW], BF16, tag="vm")
                        nc.vector.tensor_scalar_mul(vm[:], th[:], va_ap)
                        if vacc is None:
                            vacc = vm
                        else:
                            # fresh output tile: an in-place add costs ~20%
                            # more on DVE than a non-aliased one
                            nv = actp.tile([128, PW], BF16, tag="vacc")
                            nc.vector.tensor_add(nv[:], vacc[:], vm[:])
                            vacc = nv
                    pending.append((vacc, b, sbp))

                # ---- fill phase: pair 0 with per-chunk loads, W chunks
                # interleaved so the first matmul starts ~0.6us in ----
                nc.gpsimd.dma_start(w8_sb[:, :, :, :], w8_d[:, :, :, :])
                nc.scalar.dma_start(va_sb[:], va_d[:, :])
                nc.scalar.dma_start(bias_sb[:], bias_d[:, :])
                for hcc in range(HC):
                    nc.sync.dma_start(wst_sb[:, hcc * A:(hcc + 1) * A],
                                      wst_d[:, hcc * A:(hcc + 1) * A])

                # pairs 1-2 load via the scalar queue: the sync queue is
                # already committed to the W + fill chunks, and ACT has no
                # work yet so its sequencer issues these for free
                xb, x8 = load_pair(0, fill=True)
                pref = [load_pair(1, eng=nc.scalar),
                        load_pair(2, eng=nc.scalar)]
                do_pair(0, 0, xb, x8, fill=True)

                for pi in range(1, NPAIR):
                    if pi + 2 < NPAIR:
                        pref.append(load_pair(pi + 2))
                    b, sbp = divmod(pi, NSB // 2)
                    do_pair(b, sbp, *pref.pop(0), last=(pi == NPAIR - 1))

                while pending:
                    emit_reduce(*pending.pop(0))

    nc.compile()
    return nc


def _prep_host(dec_out, enc_outs, W_s, W_t, b_t, v_a):
    nac8 = len(FP8_AC) + len(HALF8_AC)
    # sort attention rows by |v_a| ascending (free relabeling of a)
    perm = np.argsort(np.abs(v_a), kind="stable")
    Wp = W_s[perm].astype(np.float64) * WSCALE
    vap = v_a[perm]
    # dec bias, exact on host: bias[a, b] = (W_t @ dec[b] + b_t)[a], permuted
    bias = (dec_out.astype(np.float64) @ W_t[perm].T.astype(np.float64)
            + b_t[perm].astype(np.float64)).T.astype(np.float32)   # (A, B)

    # W_s.T (scaled) as [128 h-part, HC * A] bf16
    wst = np.ascontiguousarray(
        Wp.T.reshape(HC, 128, A).transpose(1, 0, 2).reshape(128, HC * A)
    ).astype(BF16_NP)
    # fp8 W per fp8 a-chunk: [128 h-part, nac8, HC, 128 a]
    w8 = np.zeros((128, nac8, HC, 128), dtype=np.float64)
    for ai, ac in enumerate(tuple(FP8_AC) + tuple(HALF8_AC)):
        blk = Wp[ac * 128:(ac + 1) * 128, :]          # (128 a, H)
        w8[:, ai, :, :] = blk.T.reshape(HC, 128, 128).transpose(1, 0, 2)
    w8 = np.ascontiguousarray(w8).astype(E4M3_NP)

    va4 = np.ascontiguousarray(
        vap.reshape(AC, 128).T).astype(np.float32)           # (128, AC)

    enc_bf = enc_outs.astype(BF16_NP)                        # (S, B, H)
    enc_f8 = enc_outs.astype(E4M3_NP)
    in_maps = []
    for k in range(NCORES):
        # -> [b, sb, p, hc, c] -> row (b*NSB+sb)*128 + p, col hc*SBLK + c
        e = enc_bf[:, k * BL:(k + 1) * BL, :]
        e6 = e.reshape(NSB, SBLK, BL, HC, 128).transpose(2, 0, 4, 3, 1)
        enc_l = np.ascontiguousarray(e6).reshape(NBLK * 128, HC * SBLK)
        e8 = enc_f8[:, k * BL:(k + 1) * BL, :]
        e86 = e8.reshape(NSB, SBLK, BL, HC, 128).transpose(2, 0, 4, 3, 1)
        enc8_l = np.ascontiguousarray(e86).reshape(NBLK * 128, HC, SBLK)
        bl = bias[:, k * BL:(k + 1) * BL]                    # (A, BL)
        bias_l = np.ascontiguousarray(
            bl.reshape(AC, 128, BL).transpose(1, 0, 2).reshape(128, AC * BL))
        in_maps.append({
            "enc": enc_l,
            "enc8": enc8_l,
            "wst": wst,
            "w8": w8,
            "bias": bias_l,
            "va": va4,
        })
    return in_maps


def kernel(dec_out, enc_outs, W_s, W_t, b_t, v_a, trace=False):
    dec_out = np.asarray(dec_out)
    enc_outs = np.asarray(enc_outs)
    if "nc" not in _CACHE:
        _CACHE["nc"] = build_kernel()
    nc = _CACHE["nc"]
    in_maps = _prep_host(dec_out, enc_outs,
                         np.asarray(W_s), np.asarray(W_t),
                         np.asarray(b_t), np.asarray(v_a))
    res = run_bass_kernel_spmd(nc, in_maps, core_ids=list(range(NCORES)),
                               trace=trace)
    out = np.concatenate(
        [res.results[k]["scores"].reshape(BL, S) for k in range(NCORES)],
        axis=0).astype(np.float32)
    if trace:
        _CACHE["last_result"] = res
    return out
